# revision 42
# baseline (speedup 1.0000x reference)
"""Trainium2 Bass kernel for nn_AnchorGate (B=2048, A=1024, D=512, NN=3).

Math: the reference's per-(b,a) 6x6 Cayley-Menger determinant reduces exactly to
    raw_det = 16 * det(H_a) * (en_b - y0 - sum_i y_i^2)
with H_a the 3x3 Gram matrix of the anchor-simplex edge vectors,
y_i = e_b . Z'_ai + beta_ai  (Z' from Cholesky H = L L^T, rows of L^-1 applied
to edge vectors), y0 = 2 e.v1 - |v1|^2.  Verified to 1e-13 against the
reference in float64.

Sharding: output A-sharded (core c produces gate[:, 128c:128c+128]).  The rank
feature (argsort-argsort) is approximated by exact row counts at 49 per-row
gaussian-quantile thresholds (B-sharded) + per-element CDF interpolation via a
per-partition LUT gather; two small collectives (AllGather of count stats,
AllReduce of validity moments) connect the shardings.  End-to-end max
elementwise relative error vs the fp32 reference measured at ~8e-3 in numpy
simulation (fp16 matmuls with fp32 bias rows; knn selection in fp32).
"""
import os
import sys
import numpy as np

for _p in ("/opt/trn_rl_repo",):
    if _p not in sys.path and os.path.isdir(_p):
        sys.path.insert(0, _p)

B, A, D = 2048, 1024, 512
NCORES = 8
BSH = B // NCORES      # 256
ASL = A // NCORES      # 128
NT = 48                # NT+1 = 49 threshold knots
NK = NT + 1
ZROW = np.array([
    -2.318758010864258, -1.871870756149292, -1.635039210319519,
    -1.4652338027954102, -1.3295291662216187, -1.2146756649017334,
    -1.113937258720398, -1.0233922004699707, -0.9405436515808105,
    -0.8636956810951233, -0.7916386127471924, -0.7234755754470825,
    -0.6585199236869812, -0.5962317585945129, -0.5361762642860413,
    -0.47799599170684814, -0.4213912785053253, -0.3661063611507416,
    -0.31191906332969666, -0.2586330771446228, -0.20607197284698486,
    -0.15407446026802063, -0.10249050706624985, -0.051178012043237686, 0.0,
    0.051178012043237686, 0.10249050706624985, 0.15407446026802063,
    0.20607197284698486, 0.2586330771446228, 0.31191906332969666,
    0.3661063611507416, 0.4213912785053253, 0.47799599170684814,
    0.5361762642860413, 0.5962317585945129, 0.6585199236869812,
    0.7234755754470825, 0.7916386127471924, 0.8636956810951233,
    0.9405436515808105, 1.0233922004699707, 1.113937258720398,
    1.2146756649017334, 1.3295291662216187, 1.4652338027954102,
    1.635039210319519, 1.871870756149292, 2.318758010864258],
    dtype=np.float32).reshape(1, NK)
_V = np.polynomial.chebyshev.chebvander(2.0 * (np.arange(NK) + 0.5) / NK - 1.0, 12)
WMAT = np.ascontiguousarray(np.linalg.pinv(_V), dtype=np.float32)  # (13, 49)


class _PhaseDone(Exception):
    def __init__(self, nc):
        self.nc = nc


def build_nc(phase_limit=99):
    import concourse.bass as bass
    import concourse.tile as tile
    from concourse import bacc, mybir
    from concourse.masks import make_identity
    from contextlib import ExitStack

    F32 = mybir.dt.float32
    F16 = mybir.dt.float16
    U16 = mybir.dt.uint16
    I16 = mybir.dt.int16
    AF = mybir.ActivationFunctionType
    OP = mybir.AluOpType
    AX = mybir.AxisListType

    def sap(base_ap, parts, elems, dims):
        """Flat AP into an SBUF tile: offset by (parts, elems); dims =
        [(part_step, elem_step, num), ...] in the tile's flat element space."""
        pitch = base_ap.ap[0][0]
        return bass.AP(
            tensor=base_ap.tensor,
            offset=base_ap.offset + parts * pitch + elems,
            ap=[[ps * pitch + es, n] for (ps, es, n) in dims])

    def dap(handle, elems, dims):
        """Flat AP into a DRAM tensor; dims = [(elem_step, num), ...]."""
        return bass.AP(tensor=handle, offset=elems,
                       ap=[[s, n] for (s, n) in dims])

    nc = bacc.Bacc(None, target_bir_lowering=False)
    nc.num_devices = NCORES

    d_emb = nc.dram_tensor("embedding", [B, D], F32, kind="ExternalInput")
    d_embs = nc.dram_tensor("emb_shard", [BSH, D], F32, kind="ExternalInput")
    d_anc = nc.dram_tensor("anchors", [A, D], F32, kind="ExternalInput")
    d_ancs = nc.dram_tensor("anchors_slice", [ASL, D], F32, kind="ExternalInput")
    d_z = nc.dram_tensor("zrow", [1, NK], F32, kind="ExternalInput")
    d_wm = nc.dram_tensor("wmat", [13, NK], F32, kind="ExternalInput")
    d_w1 = nc.dram_tensor("w1", [3, 16], F32, kind="ExternalInput")
    d_b1 = nc.dram_tensor("b1", [1, 16], F32, kind="ExternalInput")
    d_w2 = nc.dram_tensor("w2", [16, 1], F32, kind="ExternalInput")
    d_b2 = nc.dram_tensor("b2", [1, 1], F32, kind="ExternalInput")
    d_out = nc.dram_tensor("out", [B, ASL], F32, kind="ExternalOutput")

    s_anc16 = nc.dram_tensor("s_anc16", [A, D], F16)
    s_ancs16 = nc.dram_tensor("s_ancs16", [ASL, D], F16)
    s_emb16 = nc.dram_tensor("s_emb16", [B, D], F16)
    s_embs16 = nc.dram_tensor("s_embs16", [BSH, D], F16)
    s_w = nc.dram_tensor("s_w", [4 * ASL, D], F16)
    s_vidx = nc.dram_tensor("s_vidx", [ASL * 4], I16)
    s_misc = nc.dram_tensor("s_misc", [2048], F32)
    cc_cs_in = nc.dram_tensor("cc_cs_in", [BSH, 52], F32)
    cc_cs_out = nc.dram_tensor("cc_cs_out", [B, 52], F32, addr_space="Shared")
    cc_vs_in = nc.dram_tensor("cc_vs_in", [B, 2], F32)
    cc_rf_in = nc.dram_tensor("cc_rf_in", [NCORES, BSH, ASL], F16)
    cc_rf_out = nc.dram_tensor("cc_rf_out", [NCORES, BSH, ASL], F16)
    cc_vs_out = nc.dram_tensor("cc_vs_out", [B, 2], F32, addr_space="Shared")

    import itertools
    _sc = itertools.count()

    with tile.TileContext(nc) as tc, ExitStack() as ctx:
        const = ctx.enter_context(tc.tile_pool(name="const", bufs=1))
        junkp = ctx.enter_context(tc.tile_pool(name="junk", bufs=2))
        sm = ctx.enter_context(tc.tile_pool(name="small", bufs=1))

        def stile(shape=(128, 1)):
            n = "sc%d" % next(_sc)
            return sm.tile(list(shape), F32, tag=n, name=n)

        def vmul(x, y):
            t = stile(x.shape); nc.vector.tensor_tensor(t[:], x, y, OP.mult); return t[:]

        def vadd(x, y):
            t = stile(x.shape); nc.vector.tensor_tensor(t[:], x, y, OP.add); return t[:]

        def vsub(x, y):
            t = stile(x.shape); nc.vector.tensor_tensor(t[:], x, y, OP.subtract); return t[:]

        def vts(x, s1, op0, s2=None, op1=OP.bypass):
            t = stile(x.shape)
            nc.vector.tensor_scalar(t[:], x, s1, s2, op0, op1)
            return t[:]

        def vrecip(x):
            t = stile(x.shape); nc.vector.reciprocal(t[:], x); return t[:]

        def vsqrt(x):
            s0 = stile(x.shape); nc.scalar.activation(s0[:], x, AF.Sqrt)
            r0 = vrecip(s0[:])
            t = vmul(x, r0)
            s1 = vadd(s0[:], t)
            return vts(s1, 0.5, OP.mult)

        ident = const.tile([128, 128], F32)
        make_identity(nc, ident[:])
        ones32 = const.tile([1, 128], F32)
        nc.vector.memset(ones32[:], 1.0)

        def vdot(acc_slice, x, y, neg=False, n=D):
            jd = junkp.tile([128, n], F32, tag="junkdot", name="jd")
            if neg:
                nc.vector.scalar_tensor_tensor(jd[:], x, -1.0, y, OP.mult, OP.mult)
            else:
                nc.vector.tensor_tensor(jd[:], x, y, OP.mult)
            nc.vector.tensor_reduce(
                acc_slice.rearrange("p (o x) -> p o x", o=1),
                jd[:].rearrange("p (o d) -> p o d", o=1), AX.X, OP.add)

        # ============ PHASE P: prep ============
        prep_cm = tc.tile_pool(name="prep", bufs=1)
        prep = prep_cm.__enter__()
        anc_sb = prep.tile([128, 8, D], F32)
        nc.sync.dma_start(out=anc_sb[:], in_=d_anc[:].rearrange("(c p) d -> p c d", p=128))
        an_pc = prep.tile([128, 8], F32)
        for c8 in range(8):
            j = junkp.tile([128, D], F32, tag="junk512")
            nc.scalar.activation(j[:], anc_sb[:, c8, :], AF.Square,
                                 accum_out=an_pc[:, c8:c8 + 1])
        an_row = prep.tile([1, A], F32)
        nc.sync.dma_start(out=dap(s_misc, 0, [(1, 128), (128, 8)]), in_=an_pc[:])
        nc.sync.dma_start(out=an_row[:], in_=dap(s_misc, 0, [(0, 1), (1, A)]))
        an_bc = prep.tile([128, A], F32)
        with tc.tile_pool(name="pp", bufs=2, space="PSUM") as pp:
            for n2 in range(2):
                ps = pp.tile([128, 512], F32, tag="mm512")
                nc.tensor.matmul(ps[:], ones32[:], an_row[:, 512 * n2:512 * n2 + 512],
                                 start=True, stop=True)
                nc.scalar.copy(an_bc[:, 512 * n2:512 * n2 + 512], ps[:])

        anc16 = prep.tile([128, 8, D], F16)
        for c8 in range(8):
            nc.vector.tensor_copy(anc16[:, c8, :], anc_sb[:, c8, :])
        nc.sync.dma_start(out=s_anc16[:].rearrange("(c p) d -> p c d", p=128),
                          in_=anc16[:])
        ancT16 = const.tile([128, 4, A], F16)
        for k in range(4):
            nc.sync.dma_start_transpose(ancT16[:, k, :],
                                        s_anc16[:, 128 * k:128 * k + 128])

        ancs_sb = prep.tile([128, D], F32)
        nc.sync.dma_start(out=ancs_sb[:], in_=d_ancs[:])
        ancs16 = prep.tile([128, D], F16)
        nc.vector.tensor_copy(ancs16[:], ancs_sb[:])
        nc.sync.dma_start(out=s_ancs16[:], in_=ancs16[:])
        ancT16sl = const.tile([128, 4, ASL], F16)
        for k in range(4):
            nc.sync.dma_start_transpose(ancT16sl[:, k, :],
                                        s_ancs16[:, 128 * k:128 * k + 128])

        enf = const.tile([128, 16], F32)
        with tc.tile_pool(name="embf", bufs=3) as embf:
            for c16 in range(16):
                et = embf.tile([128, D], F32, tag="ef32")
                nc.sync.dma_start(out=et[:], in_=d_emb[128 * c16:128 * c16 + 128, :])
                et16 = embf.tile([128, D], F16, tag="ef16")
                nc.vector.tensor_copy(et16[:], et[:])
                nc.sync.dma_start(out=s_emb16[128 * c16:128 * c16 + 128, :], in_=et16[:])
                jsq = junkp.tile([128, D], F32, tag="junk512", name="jsq")
                nc.scalar.activation(jsq[:], et[:], AF.Square,
                                     accum_out=enf[:, c16:c16 + 1])
        embT16 = const.tile([128, 4, B], F16)
        for k in range(4):
            nc.sync.dma_start_transpose(embT16[:, k, :],
                                        s_emb16[:, 128 * k:128 * k + 128])

        embs_sb = prep.tile([128, 2, D], F32)
        nc.sync.dma_start(out=embs_sb[:],
                          in_=d_embs[:].rearrange("(c p) d -> p c d", p=128))
        en_sh = const.tile([128, 2], F32)
        embs16 = prep.tile([128, 2, D], F16)
        for b2 in range(2):
            j = junkp.tile([128, D], F32, tag="junk512")
            nc.scalar.activation(j[:], embs_sb[:, b2, :], AF.Square,
                                 accum_out=en_sh[:, b2:b2 + 1])
            nc.vector.tensor_copy(embs16[:, b2, :], embs_sb[:, b2, :])
        nc.sync.dma_start(out=s_embs16[:].rearrange("(c p) d -> p c d", p=128),
                          in_=embs16[:])
        embT16sh = const.tile([128, 4, BSH], F16)
        for k in range(4):
            nc.sync.dma_start_transpose(embT16sh[:, k, :],
                                        s_embs16[:, 128 * k:128 * k + 128])

        if phase_limit <= 0:
            pr = const.tile([128, 16], F32, name="probe0")
            nc.vector.tensor_copy(pr[:], an_pc[:, 0:16])
            nc.sync.dma_start(out=d_out[0:128, 0:16], in_=pr[:])
            nc.finalize(); globals()["_EARLY"] = True
            raise _PhaseDone(nc)
        # ============ PHASE A: per-anchor slice ============
        ap_cm = tc.tile_pool(name="aside", bufs=1)
        ap_pool = ap_cm.__enter__()
        ancT32 = ap_pool.tile([128, 4, A], F32)
        ancT32sl = ap_pool.tile([128, 4, ASL], F32)
        with tc.tile_pool(name="ptp", bufs=4, space="PSUM") as ptp:
            for k in range(4):
                for c8 in range(8):
                    tp = ptp.tile([128, 128], F32, tag="tp")
                    nc.tensor.transpose(tp[:], anc_sb[:, c8, 128 * k:128 * k + 128],
                                        ident[:])
                    nc.scalar.copy(ancT32[:, k, 128 * c8:128 * c8 + 128], tp[:])
                tp = ptp.tile([128, 128], F32, tag="tp")
                nc.tensor.transpose(tp[:], ancs_sb[:, 128 * k:128 * k + 128], ident[:])
                nc.scalar.copy(ancT32sl[:, k, :], tp[:])

        key = ap_pool.tile([128, A], F32)
        with tc.tile_pool(name="pga", bufs=2, space="PSUM") as pga:
            for n2 in range(2):
                gps = pga.tile([128, 512], F32, tag="mm512")
                for k in range(4):
                    nc.tensor.matmul(gps[:], ancT32sl[:, k, :],
                                     ancT32[:, k, 512 * n2:512 * n2 + 512],
                                     start=(k == 0), stop=(k == 3))
                nc.vector.scalar_tensor_tensor(
                    key[:, 512 * n2:512 * n2 + 512], gps[:], 2.0,
                    an_bc[:, 512 * n2:512 * n2 + 512], OP.mult, OP.subtract)
        vmax8 = ap_pool.tile([128, 8], F32)
        vidx8 = ap_pool.tile([128, 8], U16)
        nc.vector.max_with_indices(vmax8[:], vidx8[:], key[:])
        vidx16 = ap_pool.tile([128, 4], I16)
        nc.vector.tensor_copy(vidx16[:], vidx8[:, 0:4])
        nc.sync.dma_start(out=dap(s_vidx, 0, [(4, 128), (1, 4)]), in_=vidx16[:])
        gidx = ap_pool.tile([128, 32], I16)
        for g in range(8):
            nc.sync.dma_start(
                out=gidx[16 * g:16 * g + 16, :],
                in_=dap(s_vidx, 0, [(4, 16), (1, 4), (64, 8)]))
        AV = ap_pool.tile([128, 4, D], F32)
        nc.gpsimd.dma_gather(AV[:], d_anc[:], gidx[:], 512, 512, D)

        U = ap_pool.tile([128, 3, D], F32)
        for i in range(3):
            nc.vector.tensor_tensor(U[:, i, :], AV[:, i + 1, :], AV[:, 0, :],
                                    OP.subtract)
        Ht = ap_pool.tile([128, 6], F32)
        pairs = [(0, 0), (0, 1), (0, 2), (1, 1), (1, 2), (2, 2)]
        for n, (i, jx) in enumerate(pairs):
            vdot(Ht[:, n:n + 1], U[:, i, :], U[:, jx, :])
        H00, H01, H02 = Ht[:, 0:1], Ht[:, 1:2], Ht[:, 2:3]
        H11, H12, H22 = Ht[:, 3:4], Ht[:, 4:5], Ht[:, 5:6]

        L11 = vsqrt(H00); iL11 = vrecip(L11)
        L21 = vmul(H01, iL11); L31 = vmul(H02, iL11)
        L22 = vsqrt(vsub(H11, vmul(L21, L21))); iL22 = vrecip(L22)
        L32 = vmul(vsub(H12, vmul(L31, L21)), iL22)
        L33 = vsqrt(vsub(vsub(H22, vmul(L31, L31)), vmul(L32, L32)))
        iL33 = vrecip(L33)
        R21 = vts(vmul(vmul(iL22, L21), iL11), -1.0, OP.mult)
        R31 = vmul(vmul(vsub(vmul(L21, L32), vmul(L31, L22)), vmul(iL11, iL22)), iL33)
        R32 = vts(vmul(vmul(L32, iL22), iL33), -1.0, OP.mult)
        m0 = vsub(vmul(H11, H22), vmul(H12, H12))
        m1 = vsub(vmul(H01, H22), vmul(H12, H02))
        m2 = vsub(vmul(H01, H12), vmul(H11, H02))
        dH = vadd(vsub(vmul(H00, m0), vmul(H01, m1)), vmul(H02, m2))
        sneg = vts(dH, -16.0, OP.mult)

        w16 = ap_pool.tile([128, 4, D], F16)
        nc.vector.tensor_scalar(w16[:, 0, :], AV[:, 0, :], 2.0, None, OP.mult)
        Zp = ap_pool.tile([128, 3, D], F32)
        nc.vector.tensor_scalar(Zp[:, 0, :], U[:, 0, :], iL11, None, OP.mult)
        nc.vector.tensor_scalar(Zp[:, 1, :], U[:, 0, :], R21, None, OP.mult)
        nc.vector.scalar_tensor_tensor(Zp[:, 1, :], U[:, 1, :], iL22, Zp[:, 1, :],
                                       OP.mult, OP.add)
        nc.vector.tensor_scalar(Zp[:, 2, :], U[:, 0, :], R31, None, OP.mult)
        nc.vector.scalar_tensor_tensor(Zp[:, 2, :], U[:, 1, :], R32, Zp[:, 2, :],
                                       OP.mult, OP.add)
        nc.vector.scalar_tensor_tensor(Zp[:, 2, :], U[:, 2, :], iL33, Zp[:, 2, :],
                                       OP.mult, OP.add)
        for i in range(3):
            nc.vector.tensor_copy(w16[:, i + 1, :], Zp[:, i, :])
        bneg = ap_pool.tile([128, 4], F32)
        vdot(bneg[:, 0:1], AV[:, 0, :], AV[:, 0, :], neg=True)
        for i in range(3):
            vdot(bneg[:, i + 1:i + 2], AV[:, 0, :], Zp[:, i, :], neg=True)

        nc.sync.dma_start(out=s_w[:].rearrange("(i p) d -> p i d", p=128), in_=w16[:])
        WT = const.tile([128, 4, 4 * ASL], F16)
        for k in range(4):
            nc.sync.dma_start_transpose(WT[:, k, :], s_w[:, 128 * k:128 * k + 128])
        bias_row = const.tile([1, 4 * ASL], F32)
        nc.sync.dma_start(out=dap(s_misc, 1024, [(1, 128), (128, 4)]), in_=bneg[:])
        nc.sync.dma_start(out=bias_row[:], in_=dap(s_misc, 1024, [(0, 1), (1, 512)]))
        sneg_row = ap_pool.tile([1, ASL], F32)
        nc.sync.dma_start(out=dap(s_misc, 1536, [(1, 128), (1, 1)]), in_=sneg)
        nc.sync.dma_start(out=sneg_row[:], in_=dap(s_misc, 1536, [(0, 1), (1, 128)]))
        sneg_bc = const.tile([128, ASL], F32)
        with tc.tile_pool(name="psb", bufs=1, space="PSUM") as psb:
            sps = psb.tile([128, 128], F32, tag="tp")
            nc.tensor.matmul(sps[:], ones32[:], sneg_row[:], start=True, stop=True)
            nc.scalar.copy(sneg_bc[:], sps[:])
        ap_cm.__exit__(None, None, None)
        prep_cm.__exit__(None, None, None)
        big = ctx.enter_context(tc.tile_pool(name="big", bufs=1))
        cos_sl = big.tile([128, 16, ASL], F16, name="cos_sl")
        vn = big.tile([128, 16, ASL], F16, name="vn")
        rf = big.tile([128, 16, ASL], F16, name="rf")
        hacc = big.tile([128, 16 * ASL], F32, name="hacc")
        hacc2 = big.tile([128, 16 * ASL], F32, name="hacc2")

        if phase_limit <= 1:
            pr = const.tile([128, 16], F32, name="probe1")
            nc.vector.tensor_copy(pr[:], Ht[:, 0:6].rearrange("p x -> p x")[:, 0:6])
            nc.sync.dma_start(out=d_out[0:128, 0:6], in_=pr[:, 0:6])
            raise _PhaseDone(nc)
        # ============ PHASE B1: rank counts (b-shard) ============
        b1_cm = tc.tile_pool(name="b1p", bufs=1)
        b1p = b1_cm.__enter__()
        xR = b1p.tile([128, 2, A], F16)
        sxp = b1p.tile([128, 4], F32)
        sxx2 = b1p.tile([128, 2], F32)
        with tc.tile_pool(name="pb1", bufs=2, space="PSUM") as pb1:
            for b2 in range(2):
                for n2 in range(2):
                    cps = pb1.tile([128, 512], F32, tag="mm512")
                    for k in range(4):
                        nc.tensor.matmul(cps[:],
                                         embT16sh[:, k, 128 * b2:128 * b2 + 128],
                                         ancT16[:, k, 512 * n2:512 * n2 + 512],
                                         start=(k == 0), stop=(k == 3))
                    nc.scalar.activation(xR[:, b2, 512 * n2:512 * n2 + 512], cps[:],
                                         AF.Copy, scale=-1.0,
                                         accum_out=sxp[:, 2 * b2 + n2:2 * b2 + n2 + 1])
                j16 = junkp.tile([128, A], F16, tag="junkA16")
                nc.scalar.activation(j16[:], xR[:, b2, :], AF.Square,
                                     accum_out=sxx2[:, b2:b2 + 1])
        mu2 = b1p.tile([128, 2], F32)
        isg2 = b1p.tile([128, 2], F32)
        th16 = b1p.tile([128, 2, NK], F32)
        zrep = b1p.tile([128, NK], F32)
        nc.sync.dma_start(out=zrep[:], in_=dap(d_z, 0, [(0, 128), (1, NK)]))
        for b2 in range(2):
            sx = vadd(sxp[:, 2 * b2:2 * b2 + 1], sxp[:, 2 * b2 + 1:2 * b2 + 2])
            nc.vector.tensor_scalar(mu2[:, b2:b2 + 1], sx, 1.0 / A, None, OP.mult)
            ex2 = vts(sxx2[:, b2:b2 + 1], 1.0 / A, OP.mult)
            var = vsub(ex2, vmul(mu2[:, b2:b2 + 1], mu2[:, b2:b2 + 1]))
            sg = vsqrt(var)
            nc.vector.reciprocal(isg2[:, b2:b2 + 1], sg)
            nc.vector.tensor_scalar(th16[:, b2, :], zrep[:], sg, mu2[:, b2:b2 + 1],
                                    OP.mult, OP.add)
        cgt = b1p.tile([128, 2 * NK], F32)
        th16f = th16[:].rearrange("p b t -> p (b t)")
        for b2 in range(2):
            for tau in range(NK):
                jm = junkp.tile([128, A], F16, tag="junkA16")
                nc.vector.tensor_scalar(
                    jm[:], xR[:, b2, :], th16f[:, NK * b2 + tau:NK * b2 + tau + 1],
                    None, OP.is_gt, OP.add,
                    accum_out=cgt[:, NK * b2 + tau:NK * b2 + tau + 1])
        casc = b1p.tile([128, 2, NK], F32)
        nc.vector.tensor_scalar(casc[:], cgt[:].rearrange("p (b t) -> p b t", b=2),
                                -1.0, float(A), OP.mult, OP.add)
        # --- rank feature eval on b-shard: theta = W @ counts, Chebyshev in s=erf ---
        wrep = b1p.tile([128, 13, NK], F32)
        nc.sync.dma_start(out=wrep[:], in_=dap(d_wm, 0, [(0, 128), (1, 13 * NK)]))
        theta = b1p.tile([128, 2, 16], F32)
        thetaf = theta[:].rearrange("p b m -> p (b m)")
        for b2 in range(2):
            for m in range(13):
                vdot(thetaf[:, 16 * b2 + m:16 * b2 + m + 1],
                     casc[:, b2, :], wrep[:, m, :], n=NK)
        rfsh = b1p.tile([128, 2, A], F16)
        tf = theta[:].rearrange("p b m -> p (b m)")
        for b2 in range(2):
            zsh = junkp.tile([128, A], F32, tag="zsh")
            nc.vector.tensor_scalar(zsh[:], xR[:, b2, :], mu2[:, b2:b2 + 1],
                                    isg2[:, b2:b2 + 1], OP.subtract, OP.mult)
            s_e = junkp.tile([128, A], F16, tag="s_e")
            nc.scalar.activation(s_e[:], zsh[:], AF.Erf, scale=0.70710678)
            acc = junkp.tile([128, A], F32, tag="acc")
            nc.vector.tensor_scalar(acc[:], s_e[:], tf[:, 16 * b2 + 1:16 * b2 + 2],
                                    tf[:, 16 * b2 + 0:16 * b2 + 1], OP.mult, OP.add)
            tm1 = junkp.tile([128, A], F16, tag="tm1")  # T_{m-1}
            nc.vector.memset(tm1[:], 1.0)
            tcu = s_e  # T_m
            for m in range(2, 13):
                tmp = junkp.tile([128, A], F16, tag="tmp%d" % (m % 2))
                nc.vector.tensor_tensor(tmp[:], s_e[:], tcu[:], OP.mult)
                tnw = junkp.tile([128, A], F16, tag="tnw%d" % (m % 2))
                nc.vector.scalar_tensor_tensor(tnw[:], tmp[:], 2.0, tm1[:],
                                               OP.mult, OP.subtract)
                nc.vector.scalar_tensor_tensor(
                    acc[:], tnw[:], tf[:, 16 * b2 + m:16 * b2 + m + 1], acc[:],
                    OP.mult, OP.add)
                tm1 = tcu
                tcu = tnw
            nc.vector.tensor_scalar(acc[:], acc[:], 1.0 / (A - 1), -0.5 / (A - 1),
                                    OP.mult, OP.add)
            nc.vector.tensor_scalar(rfsh[:, b2, :], acc[:], 0.0, 1.0, OP.max, OP.min)
        # ship to a-sharded layout: block j = cols [128j, 128j+128)
        for jj in range(8):
            nc.sync.dma_start(
                out=cc_rf_in[jj].rearrange("(c p) a -> p c a", p=128),
                in_=rfsh[:, :, 128 * jj:128 * jj + 128])


        if phase_limit <= 2:
            pr = const.tile([128, 1], F32, name="probe2")
            nc.vector.tensor_copy(pr[:], enf[:, 0:1])
            nc.sync.dma_start(out=d_out[0:128, 0:1], in_=pr[:])
            raise _PhaseDone(nc)
        # ============ PHASE B2: big planes (full B x slice) ============
        b1_cm.__exit__(None, None, None)
        nc.gpsimd.collective_compute(
            "AllToAll", OP.bypass, replica_groups=[list(range(NCORES))],
            ins=[cc_rf_in[:].opt()], outs=[cc_rf_out[:].opt()])
        nc.sync.dma_start(out=rf[:],
                          in_=cc_rf_out[:].rearrange("i (c p) a -> p (i c) a", p=128))
        b2_cm = tc.tile_pool(name="b2p", bufs=1)
        b2p = b2_cm.__enter__()
        v = b2p.tile([128, 16, ASL], F32)
        sv = const.tile([128, 16], F32)
        svv = const.tile([128, 16], F32)
        with tc.tile_pool(name="pb2", bufs=2, space="PSUM") as pb2:
            for bc in range(16):
                cps = pb2.tile([128, ASL], F32, tag="cosp")
                for k in range(4):
                    nc.tensor.matmul(cps[:], embT16[:, k, 128 * bc:128 * bc + 128],
                                     ancT16sl[:, k, :], start=(k == 0), stop=(k == 3))
                nc.scalar.copy(cos_sl[:, bc, :], cps[:])
                y0ps = pb2.tile([128, 128], F32, tag="y0")
                y3ps = pb2.tile([128, 384], F32, tag="y3")
                for k in range(4):
                    nc.tensor.matmul(y0ps[:], embT16[:, k, 128 * bc:128 * bc + 128],
                                     WT[:, k, 0:128], start=(k == 0), stop=False)
                    nc.tensor.matmul(y3ps[:], embT16[:, k, 128 * bc:128 * bc + 128],
                                     WT[:, k, 128:512], start=(k == 0), stop=False)
                nc.tensor.matmul(y0ps[:], ones32[:], bias_row[:, 0:128],
                                 start=False, stop=True)
                nc.tensor.matmul(y3ps[:], ones32[:], bias_row[:, 128:512],
                                 start=False, stop=True)
                ysq = junkp.tile([128, 384], F32, tag="ysq")
                nc.scalar.activation(ysq[:], y3ps[:], AF.Square)
                q3 = junkp.tile([128, ASL], F32, tag="q3")
                nc.vector.tensor_reduce(
                    q3[:].rearrange("p (a o) -> p a o", o=1),
                    sap(ysq[:], 0, 0, [(1, 0, 128), (0, 1, 128), (0, 128, 3)]),
                    AX.X, OP.add)
                innr = junkp.tile([128, ASL], F32, tag="innr")
                nc.vector.scalar_tensor_tensor(
                    innr[:], y0ps[:], enf[:, bc:bc + 1], q3[:],
                    OP.subtract, OP.add)
                raw = junkp.tile([128, ASL], F32, tag="raw")
                nc.vector.tensor_tensor(raw[:], innr[:], sneg_bc[:], OP.mult)
                nc.scalar.activation(v[:, bc, :], raw[:], AF.Ln,
                                     accum_out=sv[:, bc:bc + 1])
                jv = junkp.tile([128, ASL], F32, tag="raw")
                nc.scalar.activation(jv[:], v[:, bc, :], AF.Square,
                                     accum_out=svv[:, bc:bc + 1])
        vcv = cc_vs_in[:].rearrange("(c p) f -> p c f", p=128)
        nc.sync.dma_start(out=vcv[:, :, 0:1], in_=sv[:])
        nc.sync.dma_start(out=vcv[:, :, 1:2], in_=svv[:])
        nc.gpsimd.collective_compute(
            "AllReduce", OP.add, replica_groups=[list(range(NCORES))],
            ins=[cc_vs_in[:].opt()], outs=[cc_vs_out[:].opt()])
        vsf = const.tile([128, 16, 2], F32)
        nc.sync.dma_start(out=vsf[:],
                          in_=cc_vs_out[:].rearrange("(c p) f -> p c f", p=128))
        muv = const.tile([128, 16], F32)
        ivv = const.tile([128, 16], F32)
        nc.vector.tensor_scalar(muv[:], vsf[:, :, 0], 1.0 / A, None, OP.mult)
        mv2 = vmul(muv[:], muv[:])
        t1 = vts(vsf[:, :, 1], 1.0 / (A - 1), OP.mult)
        t2 = vts(mv2, float(A) / (A - 1), OP.mult)
        varv = vsub(t1, t2)
        s0 = stile((128, 16)); nc.scalar.activation(s0[:], varv, AF.Sqrt)
        r0 = vrecip(s0[:])
        tn = vmul(varv, r0)
        sgv = vadd(s0[:], tn)
        sgv = vts(sgv, 0.5, OP.mult, 1e-8, OP.max)
        nc.vector.reciprocal(ivv[:], sgv)
        for bc in range(16):
            nc.vector.tensor_scalar(vn[:, bc, :], v[:, bc, :], muv[:, bc:bc + 1],
                                    ivv[:, bc:bc + 1], OP.subtract, OP.mult)
        b2_cm.__exit__(None, None, None)

        if phase_limit <= 3:
            pr = const.tile([128, 128], F32, name="probe3")
            nc.vector.tensor_copy(pr[:], vn[:, 0, :])
            nc.sync.dma_start(out=d_out[0:128, 0:128], in_=pr[:])
            raise _PhaseDone(nc)
        # ============ PHASE B4: MLP as 16 plane-combines on DVE ============
        wsc = const.tile([128, 3, 16], F32)
        nc.sync.dma_start(out=wsc[:], in_=dap(d_w1, 0, [(0, 128), (1, 48)]))
        b1r = const.tile([128, 16], F32)
        nc.sync.dma_start(out=b1r[:], in_=dap(d_b1, 0, [(0, 128), (1, 16)]))
        w2r = const.tile([128, 16], F32)
        nc.sync.dma_start(out=w2r[:], in_=dap(d_w2, 0, [(0, 128), (1, 16)]))
        b2t = const.tile([128, 1], F32)
        nc.sync.dma_start(out=b2t[:], in_=dap(d_b2, 0, [(0, 128), (1, 1)]))
        vnf = vn[:].rearrange("p c a -> p (c a)")
        cosf = cos_sl[:].rearrange("p c a -> p (c a)")
        rff = rf[:].rearrange("p c a -> p (c a)")
        nc.vector.memset(hacc[:], 0.0)
        nc.gpsimd.memset(hacc2[:], 0.0)
        mlp_cm = tc.tile_pool(name="mlpp", bufs=2)
        mlpp = mlp_cm.__enter__()
        wscf = wsc[:].rearrange("p f j -> p (f j)")
        for j in range(16):
            eng = nc.vector
            hac = hacc if j < 11 else hacc2
            sfx = "d" if j < 11 else "g"
            t = mlpp.tile([128, 16 * ASL], F16, tag="mlpt" + sfx,
                          name="mlpt" + sfx)
            eng.tensor_scalar(t[:], vnf, wscf[:, j:j + 1], None, OP.mult)
            eng.scalar_tensor_tensor(t[:], cosf, wscf[:, 16 + j:16 + j + 1],
                                     t[:], OP.mult, OP.add)
            eng.scalar_tensor_tensor(t[:], rff, wscf[:, 32 + j:32 + j + 1],
                                     t[:], OP.mult, OP.add)
            h16 = mlpp.tile([128, 16 * ASL], F16, tag="mlph" + sfx,
                            name="mlph" + sfx)
            nc.scalar.activation(h16[:], t[:], AF.Gelu, bias=b1r[:, j:j + 1])
            eng.scalar_tensor_tensor(hac[:], h16[:], w2r[:, j:j + 1],
                                     hac[:], OP.mult, OP.add)
        nc.vector.tensor_tensor(hacc[:], hacc[:], hacc2[:], OP.add)
        mlp_cm.__exit__(None, None, None)
        nc.scalar.activation(hacc2[:], hacc[:], AF.Sigmoid, bias=b2t[:])
        nc.sync.dma_start(
            out=dap(d_out, 0, [(128, 128), (16384, 16), (1, 128)]),
            in_=hacc2[:].rearrange("p (c a) -> p c a", c=16))

    nc.finalize()
    return nc


def build_limited(phase_limit):
    try:
        return build_nc(phase_limit)
    except _PhaseDone as e:
        nc = e.nc
        if not nc.is_finalized():
            nc.finalize()
        return nc


_NC = None


def kernel(embedding, anchors, tri=None, w1=None, b1=None, w2=None, b2=None):
    global _NC
    from concourse.bass_utils import run_bass_kernel_spmd
    if _NC is None:
        _NC = build_nc()
    embedding = np.ascontiguousarray(embedding, dtype=np.float32)
    anchors = np.ascontiguousarray(anchors, dtype=np.float32)
    in_maps = []
    for c in range(NCORES):
        in_maps.append({
            "embedding": embedding,
            "emb_shard": np.ascontiguousarray(embedding[BSH * c:BSH * (c + 1)]),
            "anchors": anchors,
            "anchors_slice": np.ascontiguousarray(anchors[ASL * c:ASL * (c + 1)]),
            "zrow": ZROW,
            "wmat": WMAT,
            "w1": np.ascontiguousarray(w1, dtype=np.float32),
            "b1": np.ascontiguousarray(b1, dtype=np.float32).reshape(1, 16),
            "w2": np.ascontiguousarray(w2, dtype=np.float32),
            "b2": np.ascontiguousarray(b2, dtype=np.float32).reshape(1, 1),
        })
    res = run_bass_kernel_spmd(_NC, in_maps, core_ids=list(range(NCORES)))
    out = np.concatenate([res.results[c]["out"] for c in range(NCORES)], axis=1)
    return np.ascontiguousarray(out, dtype=np.float32)


if __name__ == "__main__":
    nc = build_nc()
    print("graph built ok:",
          sum(len(getattr(f, 'instructions', [])) for f in nc.m.functions), "instrs")


# revision 43
# speedup vs baseline: 1.0881x; 1.0881x over previous
"""Trainium2 Bass kernel for nn_AnchorGate (B=2048, A=1024, D=512, NN=3).

Math: the reference's per-(b,a) 6x6 Cayley-Menger determinant reduces exactly to
    raw_det = 16 * det(H_a) * (en_b - y0 - sum_i y_i^2)
with H_a the 3x3 Gram matrix of the anchor-simplex edge vectors,
y_i = e_b . Z'_ai + beta_ai  (Z' from Cholesky H = L L^T, rows of L^-1 applied
to edge vectors), y0 = 2 e.v1 - |v1|^2.  Verified to 1e-13 against the
reference in float64.

Sharding: output A-sharded (core c produces gate[:, 128c:128c+128]).  The rank
feature (argsort-argsort) is approximated by exact row counts at 49 per-row
gaussian-quantile thresholds (B-sharded) + per-element CDF interpolation via a
per-partition LUT gather; two small collectives (AllGather of count stats,
AllReduce of validity moments) connect the shardings.  End-to-end max
elementwise relative error vs the fp32 reference measured at ~8e-3 in numpy
simulation (fp16 matmuls with fp32 bias rows; knn selection in fp32).
"""
import os
import sys
import numpy as np

for _p in ("/opt/trn_rl_repo",):
    if _p not in sys.path and os.path.isdir(_p):
        sys.path.insert(0, _p)

B, A, D = 2048, 1024, 512
NCORES = 8
BSH = B // NCORES      # 256
ASL = A // NCORES      # 128
NT = 48                # NT+1 = 49 threshold knots
NK = NT + 1
ZROW = np.array([
    -2.318758010864258, -1.871870756149292, -1.635039210319519,
    -1.4652338027954102, -1.3295291662216187, -1.2146756649017334,
    -1.113937258720398, -1.0233922004699707, -0.9405436515808105,
    -0.8636956810951233, -0.7916386127471924, -0.7234755754470825,
    -0.6585199236869812, -0.5962317585945129, -0.5361762642860413,
    -0.47799599170684814, -0.4213912785053253, -0.3661063611507416,
    -0.31191906332969666, -0.2586330771446228, -0.20607197284698486,
    -0.15407446026802063, -0.10249050706624985, -0.051178012043237686, 0.0,
    0.051178012043237686, 0.10249050706624985, 0.15407446026802063,
    0.20607197284698486, 0.2586330771446228, 0.31191906332969666,
    0.3661063611507416, 0.4213912785053253, 0.47799599170684814,
    0.5361762642860413, 0.5962317585945129, 0.6585199236869812,
    0.7234755754470825, 0.7916386127471924, 0.8636956810951233,
    0.9405436515808105, 1.0233922004699707, 1.113937258720398,
    1.2146756649017334, 1.3295291662216187, 1.4652338027954102,
    1.635039210319519, 1.871870756149292, 2.318758010864258],
    dtype=np.float32).reshape(1, NK)
_V = np.polynomial.chebyshev.chebvander(2.0 * (np.arange(NK) + 0.5) / NK - 1.0, 8)
WMAT = np.zeros((13, NK), dtype=np.float32)
WMAT[0:9] = np.linalg.pinv(_V).astype(np.float32)


class _PhaseDone(Exception):
    def __init__(self, nc):
        self.nc = nc


def build_nc(phase_limit=99):
    import concourse.bass as bass
    import concourse.tile as tile
    from concourse import bacc, mybir
    from concourse.masks import make_identity
    from contextlib import ExitStack

    F32 = mybir.dt.float32
    F16 = mybir.dt.float16
    U16 = mybir.dt.uint16
    I16 = mybir.dt.int16
    AF = mybir.ActivationFunctionType
    OP = mybir.AluOpType
    AX = mybir.AxisListType

    def sap(base_ap, parts, elems, dims):
        """Flat AP into an SBUF tile: offset by (parts, elems); dims =
        [(part_step, elem_step, num), ...] in the tile's flat element space."""
        pitch = base_ap.ap[0][0]
        return bass.AP(
            tensor=base_ap.tensor,
            offset=base_ap.offset + parts * pitch + elems,
            ap=[[ps * pitch + es, n] for (ps, es, n) in dims])

    def dap(handle, elems, dims):
        """Flat AP into a DRAM tensor; dims = [(elem_step, num), ...]."""
        return bass.AP(tensor=handle, offset=elems,
                       ap=[[s, n] for (s, n) in dims])

    nc = bacc.Bacc(None, target_bir_lowering=False)
    nc.num_devices = NCORES

    d_emb = nc.dram_tensor("embedding", [B, D], F32, kind="ExternalInput")
    d_embs = nc.dram_tensor("emb_shard", [BSH, D], F32, kind="ExternalInput")
    d_anc = nc.dram_tensor("anchors", [A, D], F32, kind="ExternalInput")
    d_ancs = nc.dram_tensor("anchors_slice", [ASL, D], F32, kind="ExternalInput")
    d_z = nc.dram_tensor("zrow", [1, NK], F32, kind="ExternalInput")
    d_wm = nc.dram_tensor("wmat", [13, NK], F32, kind="ExternalInput")
    d_w1 = nc.dram_tensor("w1", [3, 16], F32, kind="ExternalInput")
    d_b1 = nc.dram_tensor("b1", [1, 16], F32, kind="ExternalInput")
    d_w2 = nc.dram_tensor("w2", [16, 1], F32, kind="ExternalInput")
    d_b2 = nc.dram_tensor("b2", [1, 1], F32, kind="ExternalInput")
    d_out = nc.dram_tensor("out", [B, ASL], F32, kind="ExternalOutput")

    s_anc16 = nc.dram_tensor("s_anc16", [A, D], F16)
    s_ancs16 = nc.dram_tensor("s_ancs16", [ASL, D], F16)
    s_emb16 = nc.dram_tensor("s_emb16", [B, D], F16)
    s_embs16 = nc.dram_tensor("s_embs16", [BSH, D], F16)
    s_w = nc.dram_tensor("s_w", [4 * ASL, D], F16)
    s_vidx = nc.dram_tensor("s_vidx", [ASL * 4], I16)
    s_misc = nc.dram_tensor("s_misc", [2048], F32)
    cc_cs_in = nc.dram_tensor("cc_cs_in", [BSH, 52], F32)
    cc_cs_out = nc.dram_tensor("cc_cs_out", [B, 52], F32, addr_space="Shared")
    cc_vs_in = nc.dram_tensor("cc_vs_in", [B, 2], F32)
    cc_rf_in = nc.dram_tensor("cc_rf_in", [NCORES, BSH, ASL], F16)
    cc_rf_out = nc.dram_tensor("cc_rf_out", [NCORES, BSH, ASL], F16)
    cc_vs_out = nc.dram_tensor("cc_vs_out", [B, 2], F32, addr_space="Shared")

    import itertools
    _sc = itertools.count()

    with tile.TileContext(nc) as tc, ExitStack() as ctx:
        const = ctx.enter_context(tc.tile_pool(name="const", bufs=1))
        junkp = ctx.enter_context(tc.tile_pool(name="junk", bufs=2))
        sm = ctx.enter_context(tc.tile_pool(name="small", bufs=1))

        def stile(shape=(128, 1)):
            n = "sc%d" % next(_sc)
            return sm.tile(list(shape), F32, tag=n, name=n)

        def vmul(x, y):
            t = stile(x.shape); nc.vector.tensor_tensor(t[:], x, y, OP.mult); return t[:]

        def vadd(x, y):
            t = stile(x.shape); nc.vector.tensor_tensor(t[:], x, y, OP.add); return t[:]

        def vsub(x, y):
            t = stile(x.shape); nc.vector.tensor_tensor(t[:], x, y, OP.subtract); return t[:]

        def vts(x, s1, op0, s2=None, op1=OP.bypass):
            t = stile(x.shape)
            nc.vector.tensor_scalar(t[:], x, s1, s2, op0, op1)
            return t[:]

        def vrecip(x):
            t = stile(x.shape); nc.vector.reciprocal(t[:], x); return t[:]

        def vsqrt(x):
            s0 = stile(x.shape); nc.scalar.activation(s0[:], x, AF.Sqrt)
            r0 = vrecip(s0[:])
            t = vmul(x, r0)
            s1 = vadd(s0[:], t)
            return vts(s1, 0.5, OP.mult)

        ident = const.tile([128, 128], F32)
        make_identity(nc, ident[:])
        ones32 = const.tile([1, 128], F32)
        nc.vector.memset(ones32[:], 1.0)

        def vdot(acc_slice, x, y, neg=False, n=D):
            jd = junkp.tile([128, n], F32, tag="junkdot", name="jd")
            if neg:
                nc.vector.scalar_tensor_tensor(jd[:], x, -1.0, y, OP.mult, OP.mult)
            else:
                nc.vector.tensor_tensor(jd[:], x, y, OP.mult)
            nc.vector.tensor_reduce(
                acc_slice.rearrange("p (o x) -> p o x", o=1),
                jd[:].rearrange("p (o d) -> p o d", o=1), AX.X, OP.add)

        # ============ PHASE P: prep ============
        prep_cm = tc.tile_pool(name="prep", bufs=1)
        prep = prep_cm.__enter__()
        anc_sb = prep.tile([128, 8, D], F32)
        nc.sync.dma_start(out=anc_sb[:], in_=d_anc[:].rearrange("(c p) d -> p c d", p=128))
        an_pc = prep.tile([128, 8], F32)
        for c8 in range(8):
            j = junkp.tile([128, D], F32, tag="junk512")
            nc.scalar.activation(j[:], anc_sb[:, c8, :], AF.Square,
                                 accum_out=an_pc[:, c8:c8 + 1])
        an_row = prep.tile([1, A], F32)
        nc.sync.dma_start(out=dap(s_misc, 0, [(1, 128), (128, 8)]), in_=an_pc[:])
        nc.sync.dma_start(out=an_row[:], in_=dap(s_misc, 0, [(0, 1), (1, A)]))
        an_bc = prep.tile([128, A], F32)
        with tc.tile_pool(name="pp", bufs=2, space="PSUM") as pp:
            for n2 in range(2):
                ps = pp.tile([128, 512], F32, tag="mm512")
                nc.tensor.matmul(ps[:], ones32[:], an_row[:, 512 * n2:512 * n2 + 512],
                                 start=True, stop=True)
                nc.scalar.copy(an_bc[:, 512 * n2:512 * n2 + 512], ps[:])

        anc16 = prep.tile([128, 8, D], F16)
        for c8 in range(8):
            nc.vector.tensor_copy(anc16[:, c8, :], anc_sb[:, c8, :])
        nc.sync.dma_start(out=s_anc16[:].rearrange("(c p) d -> p c d", p=128),
                          in_=anc16[:])
        ancT16 = const.tile([128, 4, A], F16)
        for k in range(4):
            nc.sync.dma_start_transpose(ancT16[:, k, :],
                                        s_anc16[:, 128 * k:128 * k + 128])

        ancs_sb = prep.tile([128, D], F32)
        nc.sync.dma_start(out=ancs_sb[:], in_=d_ancs[:])
        ancs16 = prep.tile([128, D], F16)
        nc.vector.tensor_copy(ancs16[:], ancs_sb[:])
        nc.sync.dma_start(out=s_ancs16[:], in_=ancs16[:])
        ancT16sl = const.tile([128, 4, ASL], F16)
        for k in range(4):
            nc.sync.dma_start_transpose(ancT16sl[:, k, :],
                                        s_ancs16[:, 128 * k:128 * k + 128])

        enf = const.tile([128, 16], F32)
        with tc.tile_pool(name="embf", bufs=3) as embf:
            for c16 in range(16):
                et = embf.tile([128, D], F32, tag="ef32")
                nc.sync.dma_start(out=et[:], in_=d_emb[128 * c16:128 * c16 + 128, :])
                et16 = embf.tile([128, D], F16, tag="ef16")
                nc.vector.tensor_copy(et16[:], et[:])
                nc.sync.dma_start(out=s_emb16[128 * c16:128 * c16 + 128, :], in_=et16[:])
                jsq = junkp.tile([128, D], F32, tag="junk512", name="jsq")
                nc.scalar.activation(jsq[:], et[:], AF.Square,
                                     accum_out=enf[:, c16:c16 + 1])
        embT16 = const.tile([128, 4, B], F16)
        for k in range(4):
            nc.sync.dma_start_transpose(embT16[:, k, :],
                                        s_emb16[:, 128 * k:128 * k + 128])

        embs_sb = prep.tile([128, 2, D], F32)
        nc.sync.dma_start(out=embs_sb[:],
                          in_=d_embs[:].rearrange("(c p) d -> p c d", p=128))
        en_sh = const.tile([128, 2], F32)
        embs16 = prep.tile([128, 2, D], F16)
        for b2 in range(2):
            j = junkp.tile([128, D], F32, tag="junk512")
            nc.scalar.activation(j[:], embs_sb[:, b2, :], AF.Square,
                                 accum_out=en_sh[:, b2:b2 + 1])
            nc.vector.tensor_copy(embs16[:, b2, :], embs_sb[:, b2, :])
        nc.sync.dma_start(out=s_embs16[:].rearrange("(c p) d -> p c d", p=128),
                          in_=embs16[:])
        embT16sh = const.tile([128, 4, BSH], F16)
        for k in range(4):
            nc.sync.dma_start_transpose(embT16sh[:, k, :],
                                        s_embs16[:, 128 * k:128 * k + 128])

        if phase_limit <= 0:
            pr = const.tile([128, 16], F32, name="probe0")
            nc.vector.tensor_copy(pr[:], an_pc[:, 0:16])
            nc.sync.dma_start(out=d_out[0:128, 0:16], in_=pr[:])
            nc.finalize(); globals()["_EARLY"] = True
            raise _PhaseDone(nc)
        # ============ PHASE A: per-anchor slice ============
        ap_cm = tc.tile_pool(name="aside", bufs=1)
        ap_pool = ap_cm.__enter__()
        ancT32 = ap_pool.tile([128, 4, A], F32)
        ancT32sl = ap_pool.tile([128, 4, ASL], F32)
        with tc.tile_pool(name="ptp", bufs=4, space="PSUM") as ptp:
            for k in range(4):
                for c8 in range(8):
                    tp = ptp.tile([128, 128], F32, tag="tp")
                    nc.tensor.transpose(tp[:], anc_sb[:, c8, 128 * k:128 * k + 128],
                                        ident[:])
                    nc.scalar.copy(ancT32[:, k, 128 * c8:128 * c8 + 128], tp[:])
                tp = ptp.tile([128, 128], F32, tag="tp")
                nc.tensor.transpose(tp[:], ancs_sb[:, 128 * k:128 * k + 128], ident[:])
                nc.scalar.copy(ancT32sl[:, k, :], tp[:])

        key = ap_pool.tile([128, A], F32)
        with tc.tile_pool(name="pga", bufs=2, space="PSUM") as pga:
            for n2 in range(2):
                gps = pga.tile([128, 512], F32, tag="mm512")
                for k in range(4):
                    nc.tensor.matmul(gps[:], ancT32sl[:, k, :],
                                     ancT32[:, k, 512 * n2:512 * n2 + 512],
                                     start=(k == 0), stop=(k == 3))
                nc.vector.scalar_tensor_tensor(
                    key[:, 512 * n2:512 * n2 + 512], gps[:], 2.0,
                    an_bc[:, 512 * n2:512 * n2 + 512], OP.mult, OP.subtract)
        vmax8 = ap_pool.tile([128, 8], F32)
        vidx8 = ap_pool.tile([128, 8], U16)
        nc.vector.max_with_indices(vmax8[:], vidx8[:], key[:])
        vidx16 = ap_pool.tile([128, 4], I16)
        nc.vector.tensor_copy(vidx16[:], vidx8[:, 0:4])
        nc.sync.dma_start(out=dap(s_vidx, 0, [(4, 128), (1, 4)]), in_=vidx16[:])
        gidx = ap_pool.tile([128, 32], I16)
        for g in range(8):
            nc.sync.dma_start(
                out=gidx[16 * g:16 * g + 16, :],
                in_=dap(s_vidx, 0, [(4, 16), (1, 4), (64, 8)]))
        AV = ap_pool.tile([128, 4, D], F32)
        nc.gpsimd.dma_gather(AV[:], d_anc[:], gidx[:], 512, 512, D)

        U = ap_pool.tile([128, 3, D], F32)
        for i in range(3):
            nc.vector.tensor_tensor(U[:, i, :], AV[:, i + 1, :], AV[:, 0, :],
                                    OP.subtract)
        Ht = ap_pool.tile([128, 6], F32)
        pairs = [(0, 0), (0, 1), (0, 2), (1, 1), (1, 2), (2, 2)]
        for n, (i, jx) in enumerate(pairs):
            vdot(Ht[:, n:n + 1], U[:, i, :], U[:, jx, :])
        H00, H01, H02 = Ht[:, 0:1], Ht[:, 1:2], Ht[:, 2:3]
        H11, H12, H22 = Ht[:, 3:4], Ht[:, 4:5], Ht[:, 5:6]

        L11 = vsqrt(H00); iL11 = vrecip(L11)
        L21 = vmul(H01, iL11); L31 = vmul(H02, iL11)
        L22 = vsqrt(vsub(H11, vmul(L21, L21))); iL22 = vrecip(L22)
        L32 = vmul(vsub(H12, vmul(L31, L21)), iL22)
        L33 = vsqrt(vsub(vsub(H22, vmul(L31, L31)), vmul(L32, L32)))
        iL33 = vrecip(L33)
        R21 = vts(vmul(vmul(iL22, L21), iL11), -1.0, OP.mult)
        R31 = vmul(vmul(vsub(vmul(L21, L32), vmul(L31, L22)), vmul(iL11, iL22)), iL33)
        R32 = vts(vmul(vmul(L32, iL22), iL33), -1.0, OP.mult)
        m0 = vsub(vmul(H11, H22), vmul(H12, H12))
        m1 = vsub(vmul(H01, H22), vmul(H12, H02))
        m2 = vsub(vmul(H01, H12), vmul(H11, H02))
        dH = vadd(vsub(vmul(H00, m0), vmul(H01, m1)), vmul(H02, m2))
        sneg = vts(dH, -16.0, OP.mult)

        w16 = ap_pool.tile([128, 4, D], F16)
        nc.vector.tensor_scalar(w16[:, 0, :], AV[:, 0, :], 2.0, None, OP.mult)
        Zp = ap_pool.tile([128, 3, D], F32)
        nc.vector.tensor_scalar(Zp[:, 0, :], U[:, 0, :], iL11, None, OP.mult)
        nc.vector.tensor_scalar(Zp[:, 1, :], U[:, 0, :], R21, None, OP.mult)
        nc.vector.scalar_tensor_tensor(Zp[:, 1, :], U[:, 1, :], iL22, Zp[:, 1, :],
                                       OP.mult, OP.add)
        nc.vector.tensor_scalar(Zp[:, 2, :], U[:, 0, :], R31, None, OP.mult)
        nc.vector.scalar_tensor_tensor(Zp[:, 2, :], U[:, 1, :], R32, Zp[:, 2, :],
                                       OP.mult, OP.add)
        nc.vector.scalar_tensor_tensor(Zp[:, 2, :], U[:, 2, :], iL33, Zp[:, 2, :],
                                       OP.mult, OP.add)
        for i in range(3):
            nc.vector.tensor_copy(w16[:, i + 1, :], Zp[:, i, :])
        bneg = ap_pool.tile([128, 4], F32)
        vdot(bneg[:, 0:1], AV[:, 0, :], AV[:, 0, :], neg=True)
        for i in range(3):
            vdot(bneg[:, i + 1:i + 2], AV[:, 0, :], Zp[:, i, :], neg=True)

        nc.sync.dma_start(out=s_w[:].rearrange("(i p) d -> p i d", p=128), in_=w16[:])
        WT = const.tile([128, 4, 4 * ASL], F16)
        for k in range(4):
            nc.sync.dma_start_transpose(WT[:, k, :], s_w[:, 128 * k:128 * k + 128])
        bias_row = const.tile([1, 4 * ASL], F32)
        nc.sync.dma_start(out=dap(s_misc, 1024, [(1, 128), (128, 4)]), in_=bneg[:])
        nc.sync.dma_start(out=bias_row[:], in_=dap(s_misc, 1024, [(0, 1), (1, 512)]))
        sneg_row = ap_pool.tile([1, ASL], F32)
        nc.sync.dma_start(out=dap(s_misc, 1536, [(1, 128), (1, 1)]), in_=sneg)
        nc.sync.dma_start(out=sneg_row[:], in_=dap(s_misc, 1536, [(0, 1), (1, 128)]))
        sneg_bc = const.tile([128, ASL], F32)
        with tc.tile_pool(name="psb", bufs=1, space="PSUM") as psb:
            sps = psb.tile([128, 128], F32, tag="tp")
            nc.tensor.matmul(sps[:], ones32[:], sneg_row[:], start=True, stop=True)
            nc.scalar.copy(sneg_bc[:], sps[:])
        ap_cm.__exit__(None, None, None)
        prep_cm.__exit__(None, None, None)
        big = ctx.enter_context(tc.tile_pool(name="big", bufs=1))
        cos_sl = big.tile([128, 16, ASL], F16, name="cos_sl")
        vn = big.tile([128, 16, ASL], F16, name="vn")
        rf = big.tile([128, 16, ASL], F16, name="rf")
        hacc = big.tile([128, 16 * ASL], F32, name="hacc")
        hacc2 = big.tile([128, 16 * ASL], F32, name="hacc2")

        if phase_limit <= 1:
            pr = const.tile([128, 16], F32, name="probe1")
            nc.vector.tensor_copy(pr[:], Ht[:, 0:6].rearrange("p x -> p x")[:, 0:6])
            nc.sync.dma_start(out=d_out[0:128, 0:6], in_=pr[:, 0:6])
            raise _PhaseDone(nc)
        # ============ PHASE B1: rank counts (b-shard) ============
        b1_cm = tc.tile_pool(name="b1p", bufs=1)
        b1p = b1_cm.__enter__()
        xR = b1p.tile([128, 2, A], F16)
        sxp = b1p.tile([128, 4], F32)
        sxx2 = b1p.tile([128, 2], F32)
        with tc.tile_pool(name="pb1", bufs=2, space="PSUM") as pb1:
            for b2 in range(2):
                for n2 in range(2):
                    cps = pb1.tile([128, 512], F32, tag="mm512")
                    for k in range(4):
                        nc.tensor.matmul(cps[:],
                                         embT16sh[:, k, 128 * b2:128 * b2 + 128],
                                         ancT16[:, k, 512 * n2:512 * n2 + 512],
                                         start=(k == 0), stop=(k == 3))
                    nc.scalar.activation(xR[:, b2, 512 * n2:512 * n2 + 512], cps[:],
                                         AF.Copy, scale=-1.0,
                                         accum_out=sxp[:, 2 * b2 + n2:2 * b2 + n2 + 1])
                j16 = junkp.tile([128, A], F16, tag="junkA16")
                nc.scalar.activation(j16[:], xR[:, b2, :], AF.Square,
                                     accum_out=sxx2[:, b2:b2 + 1])
        mu2 = b1p.tile([128, 2], F32)
        isg2 = b1p.tile([128, 2], F32)
        th16 = b1p.tile([128, 2, NK], F32)
        zrep = b1p.tile([128, NK], F32)
        nc.sync.dma_start(out=zrep[:], in_=dap(d_z, 0, [(0, 128), (1, NK)]))
        for b2 in range(2):
            sx = vadd(sxp[:, 2 * b2:2 * b2 + 1], sxp[:, 2 * b2 + 1:2 * b2 + 2])
            nc.vector.tensor_scalar(mu2[:, b2:b2 + 1], sx, 1.0 / A, None, OP.mult)
            ex2 = vts(sxx2[:, b2:b2 + 1], 1.0 / A, OP.mult)
            var = vsub(ex2, vmul(mu2[:, b2:b2 + 1], mu2[:, b2:b2 + 1]))
            sg = vsqrt(var)
            nc.vector.reciprocal(isg2[:, b2:b2 + 1], sg)
            nc.vector.tensor_scalar(th16[:, b2, :], zrep[:], sg, mu2[:, b2:b2 + 1],
                                    OP.mult, OP.add)
        cgt = b1p.tile([128, 2 * NK], F32)
        th16f = th16[:].rearrange("p b t -> p (b t)")
        thn = b1p.tile([128, 2 * NK], F32)
        nc.vector.tensor_scalar(thn[:], th16f, -1.0, None, OP.mult)
        NSPL = 36
        for b2 in range(2):
            for tau in range(NSPL):
                jm = junkp.tile([128, A], F16, tag="junkA16")
                nc.vector.tensor_scalar(
                    jm[:], xR[:, b2, :], th16f[:, NK * b2 + tau:NK * b2 + tau + 1],
                    None, OP.is_gt, OP.add,
                    accum_out=cgt[:, NK * b2 + tau:NK * b2 + tau + 1])
            for tau in range(NSPL, NK):
                js = junkp.tile([128, A], F16, tag="junkS16")
                nc.scalar.activation(
                    js[:], xR[:, b2, :], AF.Sign,
                    bias=thn[:, NK * b2 + tau:NK * b2 + tau + 1],
                    accum_out=cgt[:, NK * b2 + tau:NK * b2 + tau + 1])
        casc = b1p.tile([128, 2, NK], F32)
        for b2 in range(2):
            nc.vector.tensor_scalar(
                casc[:, b2, 0:NSPL], cgt[:, NK * b2:NK * b2 + NSPL],
                -1.0, float(A), OP.mult, OP.add)
            nc.vector.tensor_scalar(
                casc[:, b2, NSPL:NK], cgt[:, NK * b2 + NSPL:NK * b2 + NK],
                -0.5, float(A) / 2.0, OP.mult, OP.add)
        # --- rank feature eval on b-shard: theta = W @ counts, Chebyshev in s=erf ---
        wrep = b1p.tile([128, 13, NK], F32)
        nc.sync.dma_start(out=wrep[:], in_=dap(d_wm, 0, [(0, 128), (1, 13 * NK)]))
        theta = b1p.tile([128, 2, 16], F32)
        thetaf = theta[:].rearrange("p b m -> p (b m)")
        for b2 in range(2):
            for m in range(9):
                vdot(thetaf[:, 16 * b2 + m:16 * b2 + m + 1],
                     casc[:, b2, :], wrep[:, m, :], n=NK)
        rfsh = b1p.tile([128, 2, A], F16)
        tf = theta[:].rearrange("p b m -> p (b m)")
        for b2 in range(2):
            zsh = junkp.tile([128, A], F32, tag="zsh")
            nc.vector.tensor_scalar(zsh[:], xR[:, b2, :], mu2[:, b2:b2 + 1],
                                    isg2[:, b2:b2 + 1], OP.subtract, OP.mult)
            s_e = junkp.tile([128, A], F16, tag="s_e")
            nc.scalar.activation(s_e[:], zsh[:], AF.Erf, scale=0.70710678)
            acc = junkp.tile([128, A], F32, tag="acc")
            nc.vector.tensor_scalar(acc[:], s_e[:], tf[:, 16 * b2 + 1:16 * b2 + 2],
                                    tf[:, 16 * b2 + 0:16 * b2 + 1], OP.mult, OP.add)
            tm1 = junkp.tile([128, A], F16, tag="tm1")  # T_{m-1}
            nc.vector.memset(tm1[:], 1.0)
            tcu = s_e  # T_m
            for m in range(2, 9):
                tmp = junkp.tile([128, A], F16, tag="tmp%d" % (m % 2))
                nc.vector.tensor_tensor(tmp[:], s_e[:], tcu[:], OP.mult)
                tnw = junkp.tile([128, A], F16, tag="tnw%d" % (m % 2))
                nc.vector.scalar_tensor_tensor(tnw[:], tmp[:], 2.0, tm1[:],
                                               OP.mult, OP.subtract)
                nc.vector.scalar_tensor_tensor(
                    acc[:], tnw[:], tf[:, 16 * b2 + m:16 * b2 + m + 1], acc[:],
                    OP.mult, OP.add)
                tm1 = tcu
                tcu = tnw
            nc.vector.tensor_scalar(acc[:], acc[:], 1.0 / (A - 1), -0.5 / (A - 1),
                                    OP.mult, OP.add)
            nc.vector.tensor_scalar(rfsh[:, b2, :], acc[:], 0.0, 1.0, OP.max, OP.min)
        # ship to a-sharded layout: block j = cols [128j, 128j+128)
        for jj in range(8):
            nc.sync.dma_start(
                out=cc_rf_in[jj].rearrange("(c p) a -> p c a", p=128),
                in_=rfsh[:, :, 128 * jj:128 * jj + 128])


        if phase_limit <= 2:
            pr = const.tile([128, 1], F32, name="probe2")
            nc.vector.tensor_copy(pr[:], enf[:, 0:1])
            nc.sync.dma_start(out=d_out[0:128, 0:1], in_=pr[:])
            raise _PhaseDone(nc)
        # ============ PHASE B2: big planes (full B x slice) ============
        b1_cm.__exit__(None, None, None)
        nc.gpsimd.collective_compute(
            "AllToAll", OP.bypass, replica_groups=[list(range(NCORES))],
            ins=[cc_rf_in[:].opt()], outs=[cc_rf_out[:].opt()])
        nc.sync.dma_start(out=rf[:],
                          in_=cc_rf_out[:].rearrange("i (c p) a -> p (i c) a", p=128))
        b2_cm = tc.tile_pool(name="b2p", bufs=1)
        b2p = b2_cm.__enter__()
        v = b2p.tile([128, 16, ASL], F32)
        sv = const.tile([128, 16], F32)
        svv = const.tile([128, 16], F32)
        with tc.tile_pool(name="pb2", bufs=2, space="PSUM") as pb2:
            for bc in range(16):
                cps = pb2.tile([128, ASL], F32, tag="cosp")
                for k in range(4):
                    nc.tensor.matmul(cps[:], embT16[:, k, 128 * bc:128 * bc + 128],
                                     ancT16sl[:, k, :], start=(k == 0), stop=(k == 3))
                nc.scalar.copy(cos_sl[:, bc, :], cps[:])
                y0ps = pb2.tile([128, 128], F32, tag="y0")
                y3ps = pb2.tile([128, 384], F32, tag="y3")
                for k in range(4):
                    nc.tensor.matmul(y0ps[:], embT16[:, k, 128 * bc:128 * bc + 128],
                                     WT[:, k, 0:128], start=(k == 0), stop=False)
                    nc.tensor.matmul(y3ps[:], embT16[:, k, 128 * bc:128 * bc + 128],
                                     WT[:, k, 128:512], start=(k == 0), stop=False)
                nc.tensor.matmul(y0ps[:], ones32[:], bias_row[:, 0:128],
                                 start=False, stop=True)
                nc.tensor.matmul(y3ps[:], ones32[:], bias_row[:, 128:512],
                                 start=False, stop=True)
                ysq = junkp.tile([128, 384], F32, tag="ysq")
                nc.scalar.activation(ysq[:], y3ps[:], AF.Square)
                q3 = junkp.tile([128, ASL], F32, tag="q3")
                nc.vector.tensor_reduce(
                    q3[:].rearrange("p (a o) -> p a o", o=1),
                    sap(ysq[:], 0, 0, [(1, 0, 128), (0, 1, 128), (0, 128, 3)]),
                    AX.X, OP.add)
                innr = junkp.tile([128, ASL], F32, tag="innr")
                nc.vector.scalar_tensor_tensor(
                    innr[:], y0ps[:], enf[:, bc:bc + 1], q3[:],
                    OP.subtract, OP.add)
                raw = junkp.tile([128, ASL], F32, tag="raw")
                nc.vector.tensor_tensor(raw[:], innr[:], sneg_bc[:], OP.mult)
                nc.scalar.activation(v[:, bc, :], raw[:], AF.Ln,
                                     accum_out=sv[:, bc:bc + 1])
                jv = junkp.tile([128, ASL], F32, tag="raw")
                nc.scalar.activation(jv[:], v[:, bc, :], AF.Square,
                                     accum_out=svv[:, bc:bc + 1])
        vcv = cc_vs_in[:].rearrange("(c p) f -> p c f", p=128)
        nc.sync.dma_start(out=vcv[:, :, 0:1], in_=sv[:])
        nc.sync.dma_start(out=vcv[:, :, 1:2], in_=svv[:])
        nc.gpsimd.collective_compute(
            "AllReduce", OP.add, replica_groups=[list(range(NCORES))],
            ins=[cc_vs_in[:].opt()], outs=[cc_vs_out[:].opt()])
        vsf = const.tile([128, 16, 2], F32)
        nc.sync.dma_start(out=vsf[:],
                          in_=cc_vs_out[:].rearrange("(c p) f -> p c f", p=128))
        muv = const.tile([128, 16], F32)
        ivv = const.tile([128, 16], F32)
        nc.vector.tensor_scalar(muv[:], vsf[:, :, 0], 1.0 / A, None, OP.mult)
        mv2 = vmul(muv[:], muv[:])
        t1 = vts(vsf[:, :, 1], 1.0 / (A - 1), OP.mult)
        t2 = vts(mv2, float(A) / (A - 1), OP.mult)
        varv = vsub(t1, t2)
        s0 = stile((128, 16)); nc.scalar.activation(s0[:], varv, AF.Sqrt)
        r0 = vrecip(s0[:])
        tn = vmul(varv, r0)
        sgv = vadd(s0[:], tn)
        sgv = vts(sgv, 0.5, OP.mult, 1e-8, OP.max)
        nc.vector.reciprocal(ivv[:], sgv)
        for bc in range(16):
            nc.vector.tensor_scalar(vn[:, bc, :], v[:, bc, :], muv[:, bc:bc + 1],
                                    ivv[:, bc:bc + 1], OP.subtract, OP.mult)
        b2_cm.__exit__(None, None, None)

        if phase_limit <= 3:
            pr = const.tile([128, 128], F32, name="probe3")
            nc.vector.tensor_copy(pr[:], vn[:, 0, :])
            nc.sync.dma_start(out=d_out[0:128, 0:128], in_=pr[:])
            raise _PhaseDone(nc)
        # ============ PHASE B4: MLP as 16 plane-combines on DVE ============
        wsc = const.tile([128, 3, 16], F32)
        nc.sync.dma_start(out=wsc[:], in_=dap(d_w1, 0, [(0, 128), (1, 48)]))
        b1r = const.tile([128, 16], F32)
        nc.sync.dma_start(out=b1r[:], in_=dap(d_b1, 0, [(0, 128), (1, 16)]))
        w2r = const.tile([128, 16], F32)
        nc.sync.dma_start(out=w2r[:], in_=dap(d_w2, 0, [(0, 128), (1, 16)]))
        b2t = const.tile([128, 1], F32)
        nc.sync.dma_start(out=b2t[:], in_=dap(d_b2, 0, [(0, 128), (1, 1)]))
        vnf = vn[:].rearrange("p c a -> p (c a)")
        cosf = cos_sl[:].rearrange("p c a -> p (c a)")
        rff = rf[:].rearrange("p c a -> p (c a)")
        nc.vector.memset(hacc[:], 0.0)
        nc.gpsimd.memset(hacc2[:], 0.0)
        mlp_cm = tc.tile_pool(name="mlpp", bufs=2)
        mlpp = mlp_cm.__enter__()
        wscf = wsc[:].rearrange("p f j -> p (f j)")
        for j in range(16):
            eng = nc.vector
            hac = hacc if j < 11 else hacc2
            sfx = "d" if j < 11 else "g"
            t = mlpp.tile([128, 16 * ASL], F16, tag="mlpt" + sfx,
                          name="mlpt" + sfx)
            eng.tensor_scalar(t[:], vnf, wscf[:, j:j + 1], None, OP.mult)
            eng.scalar_tensor_tensor(t[:], cosf, wscf[:, 16 + j:16 + j + 1],
                                     t[:], OP.mult, OP.add)
            eng.scalar_tensor_tensor(t[:], rff, wscf[:, 32 + j:32 + j + 1],
                                     t[:], OP.mult, OP.add)
            h16 = mlpp.tile([128, 16 * ASL], F16, tag="mlph" + sfx,
                            name="mlph" + sfx)
            nc.scalar.activation(h16[:], t[:], AF.Gelu, bias=b1r[:, j:j + 1])
            eng.scalar_tensor_tensor(hac[:], h16[:], w2r[:, j:j + 1],
                                     hac[:], OP.mult, OP.add)
        nc.vector.tensor_tensor(hacc[:], hacc[:], hacc2[:], OP.add)
        mlp_cm.__exit__(None, None, None)
        nc.scalar.activation(hacc2[:], hacc[:], AF.Sigmoid, bias=b2t[:])
        nc.sync.dma_start(
            out=dap(d_out, 0, [(128, 128), (16384, 16), (1, 128)]),
            in_=hacc2[:].rearrange("p (c a) -> p c a", c=16))

    nc.finalize()
    return nc


def build_limited(phase_limit):
    try:
        return build_nc(phase_limit)
    except _PhaseDone as e:
        nc = e.nc
        if not nc.is_finalized():
            nc.finalize()
        return nc


_NC = None


def kernel(embedding, anchors, tri=None, w1=None, b1=None, w2=None, b2=None):
    global _NC
    from concourse.bass_utils import run_bass_kernel_spmd
    if _NC is None:
        _NC = build_nc()
    embedding = np.ascontiguousarray(embedding, dtype=np.float32)
    anchors = np.ascontiguousarray(anchors, dtype=np.float32)
    in_maps = []
    for c in range(NCORES):
        in_maps.append({
            "embedding": embedding,
            "emb_shard": np.ascontiguousarray(embedding[BSH * c:BSH * (c + 1)]),
            "anchors": anchors,
            "anchors_slice": np.ascontiguousarray(anchors[ASL * c:ASL * (c + 1)]),
            "zrow": ZROW,
            "wmat": WMAT,
            "w1": np.ascontiguousarray(w1, dtype=np.float32),
            "b1": np.ascontiguousarray(b1, dtype=np.float32).reshape(1, 16),
            "w2": np.ascontiguousarray(w2, dtype=np.float32),
            "b2": np.ascontiguousarray(b2, dtype=np.float32).reshape(1, 1),
        })
    res = run_bass_kernel_spmd(_NC, in_maps, core_ids=list(range(NCORES)))
    out = np.concatenate([res.results[c]["out"] for c in range(NCORES)], axis=1)
    return np.ascontiguousarray(out, dtype=np.float32)


if __name__ == "__main__":
    nc = build_nc()
    print("graph built ok:",
          sum(len(getattr(f, 'instructions', [])) for f in nc.m.functions), "instrs")
